# revision 1
# baseline (speedup 1.0000x reference)
"""EdgeNetworkLayer Trainium2 kernel: 8-core SPMD, edges sharded.

messages[e,i] = sum_{k,j} z[e,k] * h_w[e,j] * W2[k, i*128+j]
with z = relu(ef @ W1 + b1) computed on-device; the bilinear form is one PE
matmul chain with contraction dim (k,j) = 64*128 = 8192:
  msgT[i, e] = sum_t W2p_t[p, i].T @ PT_t[p, e]
where tile t = (g, b), partition p = (a, c), k = 4g+a, j = 32b+c,
PT_t[p, e] = z[e, 4g+a] * h_w[e, 32b+c]  (z rows DMA-replicated x32, h rows
block-copied x4, multiplied on DVE). b1 folded via ones-row in efT; b2 folded
as a 65th fp32 contraction tile with rhs = h_wT. Main matmul in float32r.
Segment-sum via band-limited one-hot matmul in fp16 (S exactly 0/1, messages
rounded to fp16) after host-sorting edges by tgt.

Edges processed in two halves: half-1's DVE-bound main phase hides half-0's
scatter and the first ReduceScatter (node rows [0, NA*128) that only half-0
edges touch). Second RS + per-core GRU (transposed layout) finish the tail;
the GRU shard of core c is rows [SA*c, SA*(c+1)) u [NA*128 + SB*c, ...+SB).

Set EXACT_FP32 = True for a full-fp32 datapath (slower, ~5e-6 rel err).
"""
import numpy as np

N, H, E, ED, MLP_HID = 8192, 128, 16384, 16, 64
NCORES = 8
ES = E // NCORES          # 2048 edges per core
EH = ES // 2              # 1024 edges per half
P = 128
ET = ES // P              # 16 edge tiles
ETH = ET // 2             # 8 per half
KG = 16                   # k-groups of 4
NS = N // NCORES          # 1024 nodes per core (GRU shard)
NT128 = N // P            # 64 global node tiles

EXACT_FP32 = False
PT_FP16 = True   # fp16 PT datapath: ~2x faster DVE, adds ~3e-3 error


def _plan(shards):
    """Band plan + half-split boundary, uniform across cores."""
    base = np.zeros(ET, np.int64)
    endv = np.zeros(ET, np.int64)
    for ti in range(ET):
        base[ti] = (min(int(shards[c][1][ti * P]) for c in range(NCORES)) // P) * P
        endv[ti] = max(int(shards[c][1][ti * P + P - 1]) for c in range(NCORES)) + 1
    W_band = int(np.max(endv - base))
    W_band = max(256, ((W_band + P - 1) // P) * P)
    W_band = min(W_band, N)
    base = np.minimum(base, N - W_band)

    contrib = [[] for _ in range(NT128)]
    for ti in range(ET):
        for ng in range(base[ti] // P, (base[ti] + W_band) // P):
            contrib[ng].append(ti)
    # NA: leading node tiles whose contributors all lie in edge half 0
    na = 0
    while na < NT128 and (not contrib[na] or max(contrib[na]) < ETH):
        na += 1
    NA = (na // 8) * 8
    NA = max(8, min(NA, NT128 - 8))
    return base, W_band, contrib, NA


def _host_prep(h, edge_index, edge_features, W1, b1, W2, b2, W_ih, W_hh, b_ih, b_hh):
    f32 = np.float32
    sdt = f32 if EXACT_FP32 else np.float16
    h = np.ascontiguousarray(h, f32)
    src_all = np.asarray(edge_index[0], np.int64)
    tgt_all = np.asarray(edge_index[1], np.int64)
    ef_all = np.asarray(edge_features, f32)

    shards = []
    for c in range(NCORES):
        sl = slice(c * ES, (c + 1) * ES)
        s, t, ef = src_all[sl], tgt_all[sl], ef_all[sl]
        order = np.argsort(t, kind="stable")
        shards.append((s[order], t[order], np.ascontiguousarray(ef[order])))

    base, W_band, contrib, NA = _plan(shards)
    for c in range(NCORES):
        t = shards[c][1]
        for ti in range(ET):
            seg = t[ti * P:(ti + 1) * P]
            assert seg.min() >= base[ti] and seg.max() < base[ti] + W_band, "band miss"

    # W2 tiles: [(g,b), (a,c), i]; b2 separately in fp32; host layout [p, t, i]
    W2r = np.asarray(W2, f32).reshape(MLP_HID, H, H)            # [k, i, j]
    W2g = W2r.reshape(KG, 4, H, 4, 32)                          # [g, a, i, b, c]
    W2p = W2g.transpose(0, 3, 1, 4, 2).reshape(64, P, H)        # [(g,b), (a,c), i]
    wdt = np.float16 if (PT_FP16 and not EXACT_FP32) else f32
    W2P_host = np.ascontiguousarray(W2p.transpose(1, 0, 2).astype(wdt))  # [p, 64, i]
    W2P32_host = np.ascontiguousarray(
        np.asarray(b2, f32).reshape(H, H).T.astype(np.float16 if (PT_FP16 and not EXACT_FP32) else f32))

    W1p = np.concatenate([np.asarray(W1, f32), np.asarray(b1, f32)[None, :]], 0)

    gdt = f32
    W_ihT = np.ascontiguousarray(np.asarray(W_ih, f32).T.astype(gdt))   # [128, 384]
    W_hhT = np.ascontiguousarray(np.asarray(W_hh, f32).T.astype(gdt))
    b_ih = np.asarray(b_ih, f32)
    b_hh = np.asarray(b_hh, f32)
    b_r = (b_ih[:H] + b_hh[:H]).reshape(H, 1).astype(f32)
    b_z = (b_ih[H:2 * H] + b_hh[H:2 * H]).reshape(H, 1).astype(f32)
    b_in = b_ih[2 * H:].reshape(H, 1).astype(f32)
    b_hn = b_hh[2 * H:].reshape(H, 1).astype(f32)

    SA = NA * (P // NCORES)              # GRU chunk-A size per core
    in_maps = []
    for c in range(NCORES):
        s, t, ef = shards[c]
        efT = np.concatenate([ef.T, np.ones((1, ES), f32)], 0)               # [17, ES]
        srcidx = np.ascontiguousarray(s.reshape(ET, P).T).astype(np.int32)   # [128, ET]
        toff = t.reshape(ET, P) - base[:, None]                              # [ET, 128]
        Sband = (np.arange(W_band)[None, None, :] == toff[:, :, None])
        Sband = np.ascontiguousarray(
            Sband.transpose(1, 0, 2).astype(sdt))                            # [128, ET, W]
        SB = NS - SA
        gru_rows = np.r_[SA * c:SA * (c + 1),
                         NA * P + SB * c:NA * P + SB * (c + 1)]
        hTs = np.ascontiguousarray(h[gru_rows].T)                            # [128, 1024]
        in_maps.append(dict(
            hfull=h, efT=efT, srcidx=srcidx, Sband=Sband, W2P=W2P_host,
            W2P32=W2P32_host, W1p=W1p, WihT=W_ihT, WhhT=W_hhT, b_r=b_r,
            b_z=b_z, b_in=b_in, b_hn=b_hn, hTs=hTs))
    return in_maps, base, W_band, contrib, NA


def _build_program(base, W_band, contrib, NA):
    import concourse.bass as bass
    import concourse.bacc as bacc
    import concourse.tile as tile
    import concourse.mybir as mybir
    from concourse.masks import make_identity

    dt = mybir.dt.float32
    dtr = dt if EXACT_FP32 else (mybir.dt.float16 if PT_FP16 else mybir.dt.float32r)
    dtz = dt if EXACT_FP32 else (mybir.dt.float16 if PT_FP16 else dt)  # z/h operand dtype
    dts = dt if EXACT_FP32 else mybir.dt.float16   # scatter dtype
    dtb2 = dtz if PT_FP16 else dt                  # b2-tile matmul dtype
    dtg = dt  # GRU matmul dtype (fp32: tail is latency-bound, fp16 saves nothing)
    dti = mybir.dt.int32
    AF = mybir.ActivationFunctionType
    OP = mybir.AluOpType

    NB = NT128 - NA
    SA = NA * (P // NCORES)   # chunk-A columns in GRU
    SB = NS - SA

    nc = bacc.Bacc("TRN2", target_bir_lowering=False, debug=False,
                   num_devices=NCORES)

    h_d = nc.dram_tensor("hfull", [N, H], dt, kind="ExternalInput")
    efT_d = nc.dram_tensor("efT", [ED + 1, ES], dt, kind="ExternalInput")
    src_d = nc.dram_tensor("srcidx", [P, ET], dti, kind="ExternalInput")
    S_d = nc.dram_tensor("Sband", [P, ET, W_band], dts, kind="ExternalInput")
    W2P_d = nc.dram_tensor("W2P", [P, 64, H], dtr, kind="ExternalInput")
    W2P32_d = nc.dram_tensor("W2P32", [P, H], dtb2, kind="ExternalInput")
    W1p_d = nc.dram_tensor("W1p", [ED + 1, MLP_HID], dt, kind="ExternalInput")
    WihT_d = nc.dram_tensor("WihT", [H, 3 * H], dtg, kind="ExternalInput")
    WhhT_d = nc.dram_tensor("WhhT", [H, 3 * H], dtg, kind="ExternalInput")
    br_d = nc.dram_tensor("b_r", [H, 1], dt, kind="ExternalInput")
    bz_d = nc.dram_tensor("b_z", [H, 1], dt, kind="ExternalInput")
    bin_d = nc.dram_tensor("b_in", [H, 1], dt, kind="ExternalInput")
    bhn_d = nc.dram_tensor("b_hn", [H, 1], dt, kind="ExternalInput")
    hTs_d = nc.dram_tensor("hTs", [H, NS], dt, kind="ExternalInput")
    out_d = nc.dram_tensor("out_hT", [H, NS], dt, kind="ExternalOutput")

    with tile.TileContext(nc) as tc:
        with (
            tc.tile_pool(name="const", bufs=1) as cp,
            tc.tile_pool(name="dram", bufs=1, space="DRAM") as dram,
            tc.tile_pool(name="work", bufs=1) as wp,
        ):
            # ---------- gathers first (they head the critical chain)
            srci = cp.tile([P, ET], dti)
            nc.sync.dma_start(srci[:], src_d[:])
            hw = wp.tile([P, ET, P], dt)
            for t in range(ET):
                nc.gpsimd.indirect_dma_start(
                    out=hw[:, t, :], out_offset=None, in_=h_d[:],
                    in_offset=bass.IndirectOffsetOnAxis(ap=srci[:, t:t + 1], axis=0))

            ident = cp.tile([P, P], dt)
            make_identity(nc, ident[:])
            efT = cp.tile([ED + 1, ES], dt)
            nc.sync.dma_start(efT[:], efT_d[:])
            W1p = cp.tile([ED + 1, MLP_HID], dt)
            nc.sync.dma_start(W1p[:], W1p_d[:])

            zT_dram = dram.tile([MLP_HID, ES], dtz)
            m_dramA = dram.tile([NA * P, H], dt)
            m_dramB = dram.tile([NB * P, H], dt)
            rs_outA = dram.tile([SA, H], dt)
            rs_outB = dram.tile([SB, H], dt)

            # ---------- phase Z: zT = relu(W1p.T @ efT)   [64, ES]
            with tc.tile_pool(name="psz", bufs=1, space="PSUM") as psz:
                zps = psz.tile([MLP_HID, ES], dt, tag="zps")
                for s in range(ES // 512):
                    nc.tensor.matmul(zps[:, s * 512:(s + 1) * 512], W1p[:],
                                     efT[:, s * 512:(s + 1) * 512],
                                     start=True, stop=True)
                zT = wp.tile([MLP_HID, ES], dtz)
                nc.scalar.activation(zT[:], zps[:], AF.Relu)
                nc.sync.dma_start(zT_dram[:], zT[:])

            # ---------- transpose h_w -> h_wT, build H32 (SBUF->SBUF DMA)
            hwT = wp.tile([P, ES], dtz)
            H32 = wp.tile([P, 4, ES], dtz)
            with tc.tile_pool(name="pst", bufs=3, space="PSUM") as pst:
                for t in range(ET):
                    tp = pst.tile([P, P], dt, tag="tp")
                    nc.tensor.transpose(tp[:], hw[:, t, :], ident[:])
                    nc.vector.tensor_copy(hwT[:, t * P:(t + 1) * P], tp[:])
            for hh in range(2):
                for b in range(4):
                    for a in range(4):
                        eng = (nc.scalar, nc.sync, nc.gpsimd)[(b * 4 + a) % 3]
                        eng.dma_start(
                            H32[32 * a:32 * a + 32, b, hh * EH:(hh + 1) * EH],
                            hwT[32 * b:32 * b + 32, hh * EH:(hh + 1) * EH])

            # ---------- main + scatter, two edge halves, pipelined
            w2t32 = wp.tile([P, H], dtb2)
            nc.sync.dma_start(w2t32[:], W2P32_d[:])
            msgTa = wp.tile([P, EH], dt, tag="msgTa")
            msgTb = wp.tile([P, EH], dt, tag="msgTb")
            msgT_h = [msgTa, msgTb]
            msga = wp.tile([P, ETH, P], dts, tag="msga")
            msgb = wp.tile([P, ETH, P], dts, tag="msgb")
            msg_h = [msga, msgb]
            s_tiles = {}
            NGB = 4

            # GRU params loaded early (DMA is idle at start)
            WihT = cp.tile([H, 3 * H], dtg)
            nc.sync.dma_start(WihT[:], WihT_d[:])
            WhhT = cp.tile([H, 3 * H], dtg)
            nc.sync.dma_start(WhhT[:], WhhT_d[:])
            b_r = cp.tile([H, 1], dt)
            nc.sync.dma_start(b_r[:], br_d[:])
            b_z = cp.tile([H, 1], dt)
            nc.sync.dma_start(b_z[:], bz_d[:])
            b_in = cp.tile([H, 1], dt)
            nc.sync.dma_start(b_in[:], bin_d[:])
            b_hn = cp.tile([H, 1], dt)
            nc.sync.dma_start(b_hn[:], bhn_d[:])
            hTs = cp.tile([H, NS], dt)
            nc.sync.dma_start(hTs[:], hTs_d[:])
            if dtg != dt:
                hTsg = cp.tile([H, NS], dtg)
                nc.scalar.copy(hTsg[:], hTs[:])
            else:
                hTsg = hTs
            mshA = wp.tile([P, SA // P, P], dt)
            mshB = wp.tile([P, SB // P, P], dt)
            mTA = wp.tile([H, SA], dtg, tag="mTA")
            mTB = wp.tile([H, SB], dtg, tag="mTB")
            out_sb = wp.tile([H, NS], dt)

            def scatter_pass(ngl, md, ngoff):
                for gi_ in range(0, len(ngl), NGB):
                    grp = ngl[gi_:gi_ + NGB]
                    st = stage.tile([P, NGB, H], dt, tag="mstage")
                    for ci, ng in enumerate(grp):
                        cs = contrib[ng]
                        if not cs:
                            nc.vector.memset(st[:, ci, :], 0.0)
                            continue
                        pm = psm.tile([P, H], dt, tag="pm")
                        for idx, ti in enumerate(cs):
                            if ti not in s_tiles:
                                stile = spool.tile([P, W_band], dts, tag="sel")
                                nc.scalar.dma_start(stile[:], S_d[:, ti, :])
                                s_tiles[ti] = stile
                            off = ng * P - int(base[ti])
                            mh = msg_h[ti // ETH]
                            nc.tensor.matmul(
                                pm[:], s_tiles[ti][:, off:off + P],
                                mh[:, ti % ETH, :],
                                start=(idx == 0), stop=(idx == len(cs) - 1))
                        nc.vector.tensor_copy(st[:, ci, :], pm[:])
                    ng0 = grp[0] - ngoff
                    nc.sync.dma_start(
                        md[ng0 * P:(ng0 + len(grp)) * P, :].rearrange(
                            "(c p) i -> p c i", p=P), st[:])

            def gru_chunk(msh, mT, cols, off, psg):
                for t in range(cols // P):
                    tp = pst2.tile([P, P], dt, tag="tp2")
                    nc.tensor.transpose(tp[:], msh[:, t, :], ident[:])
                    nc.scalar.copy(mT[:, t * P:(t + 1) * P], tp[:])
                for c0 in range(0, cols, 512):
                    cw = min(512, cols - c0)
                    csl = slice(c0, c0 + cw)
                    osl = slice(off + c0, off + c0 + cw)
                    rz_ps = psg.tile([H, 2, 512], dt, tag="rzp")
                    gin_ps = psg.tile([H, 512], dt, tag="ginp")
                    ghn_ps = psg.tile([H, 512], dt, tag="ghnp")
                    for q in range(2):
                        nc.tensor.matmul(rz_ps[:, q, :cw],
                                         WihT[:, q * H:(q + 1) * H],
                                         mT[:, csl], start=True, stop=False)
                        nc.tensor.matmul(rz_ps[:, q, :cw],
                                         WhhT[:, q * H:(q + 1) * H],
                                         hTsg[:, osl], start=False, stop=True)
                    nc.tensor.matmul(gin_ps[:, :cw], WihT[:, 2 * H:3 * H],
                                     mT[:, csl], start=True, stop=True)
                    nc.tensor.matmul(ghn_ps[:, :cw], WhhT[:, 2 * H:3 * H],
                                     hTsg[:, osl], start=True, stop=True)
                    rz = wp.tile([H, 2, 512], dt, tag="rz")
                    nc.scalar.activation(rz[:, 0, :cw], rz_ps[:, 0, :cw],
                                         AF.Sigmoid, bias=b_r[:])
                    nc.scalar.activation(rz[:, 1, :cw], rz_ps[:, 1, :cw],
                                         AF.Sigmoid, bias=b_z[:])
                    # n = tanh(gi_n + b_in + r*(gh_n + b_hn))
                    ghn = wp.tile([H, 512], dt, tag="ghn")
                    nc.scalar.activation(ghn[:, :cw], ghn_ps[:, :cw],
                                         AF.Identity, bias=b_hn[:])
                    nc.vector.tensor_mul(ghn[:, :cw], rz[:, 0, :cw], ghn[:, :cw])
                    nc.vector.tensor_add(ghn[:, :cw], ghn[:, :cw],
                                         gin_ps[:, :cw])
                    ng_ = wp.tile([H, 512], dt, tag="ng")
                    nc.scalar.activation(ng_[:, :cw], ghn[:, :cw], AF.Tanh,
                                         bias=b_in[:])
                    # hnew = n + z*(h - n)
                    dif = wp.tile([H, 512], dt, tag="dif")
                    nc.vector.tensor_sub(dif[:, :cw], hTs[:, osl], ng_[:, :cw])
                    nc.vector.tensor_mul(dif[:, :cw], rz[:, 1, :cw], dif[:, :cw])
                    nc.vector.tensor_add(out_sb[:, osl], ng_[:, :cw],
                                         dif[:, :cw])
                    nc.sync.dma_start(out_d[:, osl], out_sb[:, osl])

            with (
                tc.tile_pool(name="pst2", bufs=2, space="PSUM") as pst2,
                tc.tile_pool(name="psm", bufs=2, space="PSUM") as psm,
                tc.tile_pool(name="spool", bufs=16) as spool,
                tc.tile_pool(name="stage", bufs=4) as stage,
            ):
                for ti in range(ET):
                    stile = spool.tile([P, W_band], dts, tag="sel")
                    nc.gpsimd.dma_start(stile[:], S_d[:, ti, :])
                    s_tiles[ti] = stile
                with (
                    tc.tile_pool(name="psacc", bufs=2, space="PSUM") as psacc,
                    tc.tile_pool(name="w2pool", bufs=2) as w2pool,
                    tc.tile_pool(name="zpool", bufs=3) as zpool,
                    tc.tile_pool(name="ptpool", bufs=3) as ptpool,
                ):
                    for half in range(2):
                        esl = slice(half * EH, (half + 1) * EH)
                        acc = psacc.tile([P, EH], dt, tag="acc")
                        for g in range(KG):
                            Z32 = zpool.tile([P, EH], dtz, tag="z32")
                            for a in range(4):
                                eng = nc.sync if a % 2 == 0 else nc.scalar
                                eng.dma_start(
                                    Z32[32 * a:32 * a + 32, :],
                                    zT_dram[4 * g + a:4 * g + a + 1, esl]
                                    .broadcast_to((32, EH)))
                            w2g = w2pool.tile([P, 4, H], dtr, tag="w2t")
                            nc.sync.dma_start(w2g[:], W2P_d[:, 4 * g:4 * g + 4, :])
                            pt = ptpool.tile([P, 4, EH], dtr, tag="pt")
                            nc.vector.tensor_tensor(
                                pt[:],
                                Z32[:].unsqueeze(1).broadcast_to((P, 4, EH)),
                                H32[:, :, esl], OP.mult)
                            for b_ in range(4):
                                tw = 4 * g + b_
                                for s in range(EH // 512):
                                    nc.tensor.matmul(
                                        acc[:, s * 512:(s + 1) * 512],
                                        w2g[:, b_, :],
                                        pt[:, b_, s * 512:(s + 1) * 512],
                                        start=(tw == 0), stop=False)
                        for s in range(EH // 512):
                            nc.tensor.matmul(acc[:, s * 512:(s + 1) * 512],
                                             w2t32[:],
                                             hwT[:, half * EH + s * 512:
                                                 half * EH + (s + 1) * 512],
                                             start=False,
                                             stop=(s == EH // 512 - 1))
                        msgT = msgT_h[half]
                        for s in range(EH // 512):
                            nc.scalar.copy(msgT[:, s * 512:(s + 1) * 512],
                                           acc[:, s * 512:(s + 1) * 512])
                        msg = msg_h[half]
                        for t in range(ETH):
                            tp = pst2.tile([P, P], dt, tag="tp2")
                            nc.tensor.transpose(tp[:],
                                                msgT[:, t * P:(t + 1) * P],
                                                ident[:])
                            nc.scalar.copy(msg[:, t, :], tp[:])
                        if half == 0:
                            scatter_pass(list(range(0, NA)), m_dramA, 0)
                            nc.gpsimd.collective_compute(
                                "ReduceScatter", OP.add,
                                replica_groups=[list(range(NCORES))],
                                ins=[m_dramA[:].opt()], outs=[rs_outA[:].opt()])
                            nc.sync.dma_start(
                                mshA[:],
                                rs_outA[:].rearrange("(t p) i -> p t i", p=P))

                # main-phase pools closed: 4 PSUM banks free for the GRU
                with tc.tile_pool(name="psg", bufs=1, space="PSUM") as psg:
                    gru_chunk(mshA, mTA, SA, 0, psg)
                    scatter_pass(list(range(NA, NT128)), m_dramB, NA)
                    nc.gpsimd.collective_compute(
                        "ReduceScatter", OP.add,
                        replica_groups=[list(range(NCORES))],
                        ins=[m_dramB[:].opt()], outs=[rs_outB[:].opt()])
                    nc.sync.dma_start(
                        mshB[:], rs_outB[:].rearrange("(t p) i -> p t i", p=P))
                    gru_chunk(mshB, mTB, SB, SA, psg)

    nc.compile()
    return nc


_CACHE = {}


def _get_program(base, W_band, contrib, NA):
    key = (tuple(base), W_band, tuple(tuple(c) for c in contrib), NA)
    if key not in _CACHE:
        _CACHE[key] = _build_program(base, W_band, contrib, NA)
    return _CACHE[key]


def kernel(h, edge_index, edge_features, W1, b1, W2, b2, W_ih, W_hh, b_ih, b_hh):
    from concourse import bass_utils

    in_maps, base, W_band, contrib, NA = _host_prep(
        h, edge_index, edge_features, W1, b1, W2, b2, W_ih, W_hh, b_ih, b_hh)
    nc = _get_program(base, W_band, contrib, NA)
    res = bass_utils.run_bass_kernel_spmd(nc, in_maps, core_ids=list(range(NCORES)))
    SA = NA * (P // NCORES)
    SB = NS - SA
    out = np.empty((N, H), np.float32)
    for c in range(NCORES):
        o = res.results[c]["out_hT"].T        # [1024, H]
        out[SA * c:SA * (c + 1)] = o[:SA]
        out[NA * P + SB * c:NA * P + SB * (c + 1)] = o[SA:]
    return out



# revision 14
# speedup vs baseline: 1.3198x; 1.3198x over previous
"""EdgeNetworkLayer Trainium2 kernel: 8-core SPMD, edges sharded BY TARGET.

Core c owns nodes [c*1024, (c+1)*1024) and every edge pointing into them, so
the per-shard segment_sum is complete locally and NO collective is needed;
each core runs the GRU on its own node shard and returns it.

messages[e,i] = sum_{k,j} z[e,k] * h_w[e,j] * W2[k, i*128+j]
with z = relu(ef @ W1 + b1); the bilinear form is one PE matmul chain with
contraction dim (k,j) = 64*128 = 8192:
  msgT[i, e] = sum_t W2p_t[p, i].T @ PT_t[p, e]
where tile t = (g, b), partition p = (a, c), k = 4g+a, j = 32b+c,
PT_t[p, e] = z[e, 4g+a] * h_w[e, 32b+c] (built on DVE in fp16: z rows
DMA-replicated x32 per (chunk, g), h rows block-copied x4 into H32).
b1 folded via ones-row in efT; b2 folded as a 65th fp16 tile w/ rhs = h_wT.

Edges are processed in 512-column chunks so PSUM accumulators stay within a
bank and the scatter + GRU for early node tiles pipeline behind later chunks'
main compute. DVE does ONLY the PT products (the critical 2x-fp16 work);
PSUM->SBUF copies run on Scalar, GRU elementwise on GpSimd.

Scatter: per node tile, PSUM-chained matmuls with stationary = msg tile
(fp16, from a PE transpose of msgT) and moving = band-limited one-hot S
(exactly 0/1 in fp16), producing mT [i, node] directly.

Z (edge MLP layer 1) and GRU matmuls run in float32r (1 cyc/row at >=256
cols, ~fp32 precision). Main matmul fp16.
"""
import numpy as np

N, H, E, ED, MLP_HID = 8192, 128, 16384, 16, 64
NCORES = 8
P = 128
NS = N // NCORES          # 1024 nodes per core
NT = NS // P              # 8 node tiles per core
KG = 16                   # k-groups of 4
GRU_COLS = 256            # GRU column-group width (>=256 for f32r fast path)


def _host_prep(h, edge_index, edge_features, W1, b1, W2, b2, W_ih, W_hh, b_ih, b_hh):
    f32, f16 = np.float32, np.float16
    h = np.ascontiguousarray(h, f32)
    src_all = np.asarray(edge_index[0], np.int64)
    tgt_all = np.asarray(edge_index[1], np.int64)
    ef_all = np.asarray(edge_features, f32)

    order = np.argsort(tgt_all, kind="stable")
    s_s, t_s, ef_s = src_all[order], tgt_all[order], ef_all[order]
    shard_of = t_s // NS
    shards = []
    for c in range(NCORES):
        m = shard_of == c
        shards.append((s_s[m], t_s[m] - c * NS, ef_s[m]))
    CAP = ((max(len(s[0]) for s in shards) + P - 1) // P) * P
    ET = CAP // P

    # tile band plan, uniform across cores
    base = np.zeros(ET, np.int64)
    endv = np.zeros(ET, np.int64)
    any_real = np.zeros(ET, bool)
    for ti in range(ET):
        lo, hi = NS, 0
        for c in range(NCORES):
            seg = shards[c][1][ti * P:(ti + 1) * P]
            if len(seg):
                any_real[ti] = True
                lo = min(lo, int(seg.min()))
                hi = max(hi, int(seg.max()) + 1)
        if any_real[ti]:
            base[ti] = (lo // P) * P
            endv[ti] = hi
    W_band = P
    for ti in range(ET):
        if any_real[ti]:
            W_band = max(W_band, int(-((base[ti] - endv[ti]) // P)) * P)

    # contrib[ng] = edge tiles feeding node tile ng (union over cores)
    contrib = [[] for _ in range(NT)]
    for ti in range(ET):
        if not any_real[ti]:
            continue
        ngs = set()
        for c in range(NCORES):
            seg = shards[c][1][ti * P:(ti + 1) * P]
            if len(seg):
                ngs |= set(int(x) for x in np.unique(seg // P))
        for ng in sorted(ngs):
            contrib[ng].append(ti)

    hf16 = np.ascontiguousarray(h.astype(f16))
    W2r = np.asarray(W2, f32).reshape(MLP_HID, H, H)            # [k, i, j]
    W2g = W2r.reshape(KG, 4, H, 4, 32)                          # [g, a, i, b, c]
    W2p = W2g.transpose(0, 3, 1, 4, 2).reshape(64, P, H)        # [(g,b), (a,c), i]
    W2P_host = np.ascontiguousarray(W2p.transpose(1, 0, 2).astype(f16))  # [p, 64, i]
    W2b2_host = np.ascontiguousarray(np.asarray(b2, f32).reshape(H, H).T.astype(f16))
    W1p = np.concatenate([np.asarray(W1, f32), np.asarray(b1, f32)[None, :]], 0)

    W_ihT = np.ascontiguousarray(np.asarray(W_ih, f32).T)       # [128, 384]
    W_hhT = np.ascontiguousarray(np.asarray(W_hh, f32).T)
    b_ih = np.asarray(b_ih, f32)
    b_hh = np.asarray(b_hh, f32)
    b_r = (b_ih[:H] + b_hh[:H]).reshape(H, 1).astype(f32)
    b_z = (b_ih[H:2 * H] + b_hh[H:2 * H]).reshape(H, 1).astype(f32)
    b_in = b_ih[2 * H:].reshape(H, 1).astype(f32)
    b_hn = b_hh[2 * H:].reshape(H, 1).astype(f32)

    in_maps = []
    for c in range(NCORES):
        s, toff, ef = shards[c]
        n = len(s)
        s_pad = np.zeros(CAP, np.int32)
        s_pad[:n] = s
        ef_pad = np.zeros((CAP, ED), f32)
        ef_pad[:n] = ef
        efT = np.concatenate([ef_pad.T, np.ones((1, CAP), f32)], 0)   # [17, CAP]
        srcidx = np.ascontiguousarray(s_pad.reshape(ET, P).T)         # [128, ET]
        Sband = np.zeros((P, ET, W_band), f16)
        idx = np.arange(n)
        Sband[idx % P, idx // P, toff - base[idx // P]] = 1.0
        hTs = np.ascontiguousarray(h[c * NS:(c + 1) * NS].T)          # [128, 1024]
        in_maps.append(dict(
            hf16=hf16, efT=efT, srcidx=srcidx, Sband=Sband, W2P=W2P_host,
            W2b2=W2b2_host, W1p=W1p, WihT=W_ihT, WhhT=W_hhT, b_r=b_r,
            b_z=b_z, b_in=b_in, b_hn=b_hn, hTs=hTs))
    return (in_maps, CAP, W_band, tuple(int(b) for b in base),
            tuple(tuple(cc) for cc in contrib))


def _build_program(CAP, W_band, base_arr, contrib):
    import concourse.bass as bass
    import concourse.bacc as bacc
    import concourse.tile as tile
    import concourse.mybir as mybir
    from concourse.masks import make_identity

    dt = mybir.dt.float32
    f16 = mybir.dt.float16
    f32r = mybir.dt.float32r
    dti = mybir.dt.int32
    AF = mybir.ActivationFunctionType
    OP = mybir.AluOpType

    ET = CAP // P
    # edge chunks of <=4 tiles (512 cols, one PSUM bank for the accumulator)
    chunks = []
    t0 = 0
    while t0 < ET:
        nt_ = min(4, ET - t0)
        chunks.append((t0, nt_))
        t0 += nt_
    NCH = len(chunks)
    last_tile_of_chunk = [t0 + nt_ - 1 for (t0, nt_) in chunks]

    # node tile ng becomes scatterable after the chunk holding max(contrib)
    ready = [[] for _ in range(NCH)]
    empty_ng = []
    for ng in range(NT):
        if not contrib[ng]:
            empty_ng.append(ng)
            continue
        need = max(contrib[ng])
        for ci in range(NCH):
            if need <= last_tile_of_chunk[ci]:
                ready[ci].append(ng)
                break
    # GRU groups of GRU_COLS columns, ready when all their node tiles are
    ngrp = GRU_COLS // P
    ng_done_at = {}
    for ci in range(NCH):
        for ng in ready[ci]:
            ng_done_at[ng] = ci
    for ng in empty_ng:
        ng_done_at[ng] = 0
    gru_ready = [[] for _ in range(NCH)]
    for gg in range(NS // GRU_COLS):
        ci = max(ng_done_at[gg * ngrp + i] for i in range(ngrp))
        gru_ready[ci].append(gg)

    nc = bacc.Bacc("TRN2", target_bir_lowering=False, debug=False,
                   num_devices=NCORES)

    hf16_d = nc.dram_tensor("hf16", [N, H], f16, kind="ExternalInput")
    efT_d = nc.dram_tensor("efT", [ED + 1, CAP], f32r, kind="ExternalInput")
    src_d = nc.dram_tensor("srcidx", [P, ET], dti, kind="ExternalInput")
    S_d = nc.dram_tensor("Sband", [P, ET, W_band], f16, kind="ExternalInput")
    W2P_d = nc.dram_tensor("W2P", [P, 64, H], f16, kind="ExternalInput")
    W2b2_d = nc.dram_tensor("W2b2", [P, H], f16, kind="ExternalInput")
    W1p_d = nc.dram_tensor("W1p", [ED + 1, MLP_HID], f32r, kind="ExternalInput")
    WihT_d = nc.dram_tensor("WihT", [H, 3 * H], f32r, kind="ExternalInput")
    WhhT_d = nc.dram_tensor("WhhT", [H, 3 * H], f32r, kind="ExternalInput")
    br_d = nc.dram_tensor("b_r", [H, 1], dt, kind="ExternalInput")
    bz_d = nc.dram_tensor("b_z", [H, 1], dt, kind="ExternalInput")
    bin_d = nc.dram_tensor("b_in", [H, 1], dt, kind="ExternalInput")
    bhn_d = nc.dram_tensor("b_hn", [H, 1], dt, kind="ExternalInput")
    hTs_d = nc.dram_tensor("hTs", [H, NS], f32r, kind="ExternalInput")
    out_d = nc.dram_tensor("out_hT", [H, NS], dt, kind="ExternalOutput")

    with tile.TileContext(nc) as tc:
        with (
            tc.tile_pool(name="const", bufs=1) as cp,
            tc.tile_pool(name="dram", bufs=1, space="DRAM") as dram,
            tc.tile_pool(name="work", bufs=1) as wp,
        ):
            # ---------- startup loads
            srci = cp.tile([P, ET], dti)
            nc.sync.dma_start(srci[:], src_d[:])
            ident16 = cp.tile([P, P], f16)
            make_identity(nc, ident16[:])
            efT = cp.tile([ED + 1, CAP], f32r)
            nc.scalar.dma_start(efT[:], efT_d[:])
            W1p = cp.tile([ED + 1, MLP_HID], f32r)
            nc.scalar.dma_start(W1p[:], W1p_d[:])

            hw16 = wp.tile([P, ET, P], f16)
            for t in range(ET):
                nc.gpsimd.indirect_dma_start(
                    out=hw16[:, t, :], out_offset=None, in_=hf16_d[:],
                    in_offset=bass.IndirectOffsetOnAxis(ap=srci[:, t:t + 1], axis=0))

            # W2 tiles: quarters on the scalar ring (needed from g=0 onward)
            W2P = cp.tile([P, 64, H], f16)
            for q in range(4):
                nc.scalar.dma_start(W2P[:, q * 16:(q + 1) * 16, :],
                                    W2P_d[:, q * 16:(q + 1) * 16, :])
            w2b2 = cp.tile([P, H], f16)
            nc.sync.dma_start(w2b2[:], W2b2_d[:])

            # GRU params early (rings are otherwise idle at t=0 on sync)
            WihT = cp.tile([H, 3 * H], f32r)
            nc.sync.dma_start(WihT[:], WihT_d[:])
            WhhT = cp.tile([H, 3 * H], f32r)
            nc.sync.dma_start(WhhT[:], WhhT_d[:])
            b_r = cp.tile([H, 1], dt)
            nc.sync.dma_start(b_r[:], br_d[:])
            b_z = cp.tile([H, 1], dt)
            nc.sync.dma_start(b_z[:], bz_d[:])
            b_in = cp.tile([H, 1], dt)
            nc.sync.dma_start(b_in[:], bin_d[:])
            b_hn = cp.tile([H, 1], dt)
            nc.sync.dma_start(b_hn[:], bhn_d[:])
            hTs = cp.tile([H, NS], f32r)
            nc.sync.dma_start(hTs[:], hTs_d[:])

            # S tiles on gpsimd after the gathers
            Sband = cp.tile([P, ET, W_band], f16)
            for t in range(ET):
                nc.gpsimd.dma_start(Sband[:, t, :], S_d[:, t, :])

            # ---------- Z phase: zT = relu(W1p.T @ efT) in fp16, f32r matmul
            # (bounced through DRAM: SBUF sources can't partition-broadcast)
            zT = wp.tile([MLP_HID, CAP], f16)
            zT_dram = dram.tile([MLP_HID, CAP], f16)
            with tc.tile_pool(name="psz", bufs=2, space="PSUM") as psz:
                for (t0_, nt_) in chunks:
                    c0, cw = t0_ * P, nt_ * P
                    zps = psz.tile([MLP_HID, 512], dt, tag="zps")
                    nc.tensor.matmul(zps[:, :cw], W1p[:],
                                     efT[:, c0:c0 + cw],
                                     start=True, stop=True)
                    nc.scalar.activation(zT[:, c0:c0 + cw], zps[:, :cw], AF.Relu)
                    nc.scalar.dma_start(zT_dram[:, c0:c0 + cw], zT[:, c0:c0 + cw])

            # ---------- h_w transposes -> hwT (fp16)
            hwT = wp.tile([P, CAP], f16)
            with tc.tile_pool(name="pst", bufs=3, space="PSUM") as pst:
                for t in range(ET):
                    tp = pst.tile([P, P], f16, tag="tp")
                    nc.tensor.transpose(tp[:], hw16[:, t, :], ident16[:])
                    nc.scalar.copy(hwT[:, t * P:(t + 1) * P], tp[:])

            # H32[p=(a,c), b, e] = h_w[e, 32b+c]: wave 1 covers chunk-0 cols
            # (only needs the first 4 gathers), wave 2 the rest.
            H32 = wp.tile([P, 4, CAP], f16)
            c0w = chunks[0][1] * P
            for b in range(4):
                for a in range(4):
                    eng = (nc.sync, nc.scalar)[(b * 4 + a) % 2]
                    eng.dma_start(H32[32 * a:32 * a + 32, b, :c0w],
                                  hwT[32 * b:32 * b + 32, :c0w])
            for b in range(4):
                for a in range(4):
                    eng = (nc.sync, nc.scalar, nc.gpsimd)[(b * 4 + a) % 3]
                    eng.dma_start(H32[32 * a:32 * a + 32, b, c0w:],
                                  hwT[32 * b:32 * b + 32, c0w:])

            # ---------- main pipeline
            msgT16 = wp.tile([P, CAP], f16)
            msg = wp.tile([P, ET, P], f16)
            mT = wp.tile([H, NS], f32r)
            out_sb = wp.tile([H, NS], dt)
            for ng in empty_ng:
                nc.gpsimd.memset(mT[:, ng * P:(ng + 1) * P], 0.0)

            def gru_group(gg, psg):
                c0 = gg * GRU_COLS
                cw = GRU_COLS
                csl = slice(c0, c0 + cw)
                rz_ps = psg.tile([H, 2, GRU_COLS], dt, tag="rzp")
                nn_ps = psg.tile([H, 2, GRU_COLS], dt, tag="nnp")
                gin_ps = nn_ps[:, 0, :]
                ghn_ps = nn_ps[:, 1, :]
                mTr = mT[:, csl]
                hTr = hTs[:, csl]
                for q in range(2):
                    nc.tensor.matmul(rz_ps[:, q, :],
                                     WihT[:, q * H:(q + 1) * H],
                                     mTr, start=True, stop=False)
                    nc.tensor.matmul(rz_ps[:, q, :],
                                     WhhT[:, q * H:(q + 1) * H],
                                     hTr, start=False, stop=True)
                nc.tensor.matmul(gin_ps, WihT[:, 2 * H:3 * H],
                                 mTr, start=True, stop=True)
                nc.tensor.matmul(ghn_ps, WhhT[:, 2 * H:3 * H],
                                 hTr, start=True, stop=True)
                rz = wp.tile([H, 2, GRU_COLS], dt, tag=f"rz{gg % 2}")
                nc.scalar.activation(rz[:, 0, :], rz_ps[:, 0, :],
                                     AF.Sigmoid, bias=b_r[:])
                nc.scalar.activation(rz[:, 1, :], rz_ps[:, 1, :],
                                     AF.Sigmoid, bias=b_z[:])
                # n = tanh(gi_n + b_in + r*(gh_n + b_hn))
                ghn = wp.tile([H, GRU_COLS], dt, tag=f"ghn{gg % 2}")
                nc.scalar.activation(ghn[:], ghn_ps, AF.Identity, bias=b_hn[:])
                nc.gpsimd.tensor_mul(ghn[:], rz[:, 0, :], ghn[:])
                nc.vector.tensor_add(ghn[:], ghn[:], gin_ps)
                ng_ = wp.tile([H, GRU_COLS], dt, tag=f"ng{gg % 2}")
                nc.scalar.activation(ng_[:], ghn[:], AF.Tanh, bias=b_in[:])
                # hnew = n + z*(h - n)
                dif = wp.tile([H, GRU_COLS], dt, tag=f"dif{gg % 2}")
                nc.gpsimd.tensor_sub(dif[:], hTs[:, csl].bitcast(dt), ng_[:])
                nc.gpsimd.tensor_mul(dif[:], rz[:, 1, :], dif[:])
                nc.gpsimd.tensor_add(out_sb[:, csl], ng_[:], dif[:])
                nc.sync.dma_start(out_d[:, csl], out_sb[:, csl])

            with (
                tc.tile_pool(name="psacc", bufs=2, space="PSUM") as psacc,
                tc.tile_pool(name="pstr", bufs=2, space="PSUM") as pstr,
                tc.tile_pool(name="psm", bufs=2, space="PSUM") as psm,
                tc.tile_pool(name="psg", bufs=1, space="PSUM") as psg,
                tc.tile_pool(name="zpool", bufs=4) as zpool,
                tc.tile_pool(name="ptpool", bufs=3) as ptpool,
            ):
                deferred = []       # (kind, payload) from previous chunk
                for ci, (t0_, nt_) in enumerate(chunks):
                    c0, cw = t0_ * P, nt_ * P
                    acc = psacc.tile([P, 512], dt, tag="acc")
                    for g in range(KG):
                        Z32 = zpool.tile([P, 512], f16, tag="z32")
                        for a in range(4):
                            eng = (nc.sync, nc.scalar)[a % 2] if ci < 2 else \
                                  (nc.sync, nc.scalar, nc.gpsimd)[(g + a) % 3]
                            eng.dma_start(
                                Z32[32 * a:32 * a + 32, :cw],
                                zT_dram[4 * g + a:4 * g + a + 1, c0:c0 + cw]
                                .broadcast_to((32, cw)))
                        pt = ptpool.tile([P, 4, 512], f16, tag="pt")
                        nc.vector.tensor_tensor(
                            pt[:, :, :cw],
                            Z32[:, :cw].unsqueeze(1).broadcast_to((P, 4, cw)),
                            H32[:, :, c0:c0 + cw], OP.mult)
                        for b in range(4):
                            nc.tensor.matmul(
                                acc[:, :cw], W2P[:, 4 * g + b, :],
                                pt[:, b, :cw],
                                start=(g == 0 and b == 0), stop=False)
                    nc.tensor.matmul(acc[:, :cw], w2b2[:],
                                     hwT[:, c0:c0 + cw],
                                     start=False, stop=True)
                    nc.scalar.copy(msgT16[:, c0:c0 + cw], acc[:, :cw])

                    # run the PREVIOUS chunk's scatter/GRU behind this chunk's
                    # main compute, then queue this chunk's transposes.
                    for kind, payload in deferred:
                        if kind == "scat":
                            ng = payload
                            pm = psm.tile([P, P], dt, tag="pm")
                            for idx, ti in enumerate(contrib[ng]):
                                off = ng * P - int(base_arr[ti])
                                nc.tensor.matmul(
                                    pm[:], msg[:, ti, :],
                                    Sband[:, ti, off:off + P],
                                    start=(idx == 0),
                                    stop=(idx == len(contrib[ng]) - 1))
                            nc.scalar.copy(mT[:, ng * P:(ng + 1) * P], pm[:])
                        else:
                            gru_group(payload, psg)
                    deferred = []
                    for t in range(t0_, t0_ + nt_):
                        tp = pstr.tile([P, P], f16, tag="tp")
                        nc.tensor.transpose(
                            tp[:], msgT16[:, t * P:(t + 1) * P], ident16[:])
                        nc.scalar.copy(msg[:, t, :], tp[:])
                    deferred += [("scat", ng) for ng in ready[ci]]
                    deferred += [("gru", gg) for gg in gru_ready[ci]]
                # tail
                for kind, payload in deferred:
                    if kind == "scat":
                        ng = payload
                        pm = psm.tile([P, P], dt, tag="pm")
                        for idx, ti in enumerate(contrib[ng]):
                            off = ng * P - int(base_arr[ti])
                            nc.tensor.matmul(
                                pm[:], msg[:, ti, :],
                                Sband[:, ti, off:off + P],
                                start=(idx == 0),
                                stop=(idx == len(contrib[ng]) - 1))
                        nc.scalar.copy(mT[:, ng * P:(ng + 1) * P], pm[:])
                    else:
                        gru_group(payload, psg)

    nc.compile()
    return nc


_CACHE = {}


def _get_program(CAP, W_band, base, contrib):
    key = (CAP, W_band, base, contrib)
    if key not in _CACHE:
        _CACHE[key] = _build_program(CAP, W_band, base, contrib)
    return _CACHE[key]


def kernel(h, edge_index, edge_features, W1, b1, W2, b2, W_ih, W_hh, b_ih, b_hh):
    from concourse import bass_utils

    in_maps, CAP, W_band, base, contrib = _host_prep(
        h, edge_index, edge_features, W1, b1, W2, b2, W_ih, W_hh, b_ih, b_hh)
    nc = _get_program(CAP, W_band, base, contrib)
    res = bass_utils.run_bass_kernel_spmd(nc, in_maps, core_ids=list(range(NCORES)))
    out = np.empty((N, H), np.float32)
    for c in range(NCORES):
        out[c * NS:(c + 1) * NS] = res.results[c]["out_hT"].T
    return out


# revision 20
# speedup vs baseline: 1.4542x; 1.1018x over previous
"""EdgeNetworkLayer Trainium2 kernel: 8-core SPMD, edges sharded BY TARGET.

Core c owns nodes [c*1024, (c+1)*1024) and every edge pointing into them, so
the per-shard segment_sum is complete locally and NO collective is needed;
each core runs the GRU on its own node shard and returns it.

messages[e,i] = sum_{k,j} z[e,k] * h_w[e,j] * W2[k, i*128+j]
with z = relu(ef @ W1 + b1); the bilinear form is one PE matmul chain with
contraction dim (k,j) = 64*128 = 8192:
  msgT[i, e] = sum_t W2p_t[p, i].T @ PT_t[p, e]
where tile t = (g, b), partition p = (a, c), k = 4g+a, j = 32b+c,
PT_t[p, e] = z[e, 4g+a] * h_w[e, 32b+c] (built on DVE in fp16).

Replication DMAs are fused along free dims (DMA ring cost is ~0.6us per op
regardless of size): Z32 is one op per (chunk, a) covering all 16 g-rows,
H32 one op per (chunk, a) from a DRAM bounce of hwT. Both z and hwT bounce
through DRAM because SBUF sources cannot partition-broadcast.

Edges are processed in 512-column chunks; scatter + GRU + h_w transposes are
interleaved one-op-per-g into the main loop's PE gaps so the PE stays
continuously busy (p-state) while DVE (the PT products) is the pacing engine.

Scatter: per node tile, PSUM-chained matmuls with stationary = msg tile
(fp16, PE transpose of msgT) and moving = band-limited one-hot S (exact 0/1
fp16), producing mT [i, node] directly. Z and GRU matmuls in float32r.
"""
import numpy as np

N, H, E, ED, MLP_HID = 8192, 128, 16384, 16, 64
NCORES = 8
P = 128
NS = N // NCORES          # 1024 nodes per core
NT = NS // P              # 8 node tiles per core
KG = 16                   # k-groups of 4
GRU_COLS = 256            # GRU column-group width (>=256 for f32r fast path)


def _host_prep(h, edge_index, edge_features, W1, b1, W2, b2, W_ih, W_hh, b_ih, b_hh):
    f32, f16 = np.float32, np.float16
    h = np.ascontiguousarray(h, f32)
    src_all = np.asarray(edge_index[0], np.int64)
    tgt_all = np.asarray(edge_index[1], np.int64)
    ef_all = np.asarray(edge_features, f32)

    order = np.argsort(tgt_all, kind="stable")
    s_s, t_s, ef_s = src_all[order], tgt_all[order], ef_all[order]
    shard_of = t_s // NS
    shards = []
    for c in range(NCORES):
        m = shard_of == c
        shards.append((s_s[m], t_s[m] - c * NS, ef_s[m]))
    CAP = ((max(len(s[0]) for s in shards) + P - 1) // P) * P
    ET = CAP // P

    # tile band plan, uniform across cores
    base = np.zeros(ET, np.int64)
    endv = np.zeros(ET, np.int64)
    any_real = np.zeros(ET, bool)
    for ti in range(ET):
        lo, hi = NS, 0
        for c in range(NCORES):
            seg = shards[c][1][ti * P:(ti + 1) * P]
            if len(seg):
                any_real[ti] = True
                lo = min(lo, int(seg.min()))
                hi = max(hi, int(seg.max()) + 1)
        if any_real[ti]:
            base[ti] = (lo // P) * P
            endv[ti] = hi
    W_band = P
    for ti in range(ET):
        if any_real[ti]:
            W_band = max(W_band, int(-((base[ti] - endv[ti]) // P)) * P)

    # contrib[ng] = edge tiles feeding node tile ng (union over cores)
    contrib = [[] for _ in range(NT)]
    for ti in range(ET):
        if not any_real[ti]:
            continue
        ngs = set()
        for c in range(NCORES):
            seg = shards[c][1][ti * P:(ti + 1) * P]
            if len(seg):
                ngs |= set(int(x) for x in np.unique(seg // P))
        for ng in sorted(ngs):
            contrib[ng].append(ti)

    hf16 = np.ascontiguousarray(h.astype(f16))
    W2r = np.asarray(W2, f32).reshape(MLP_HID, H, H)            # [k, i, j]
    W2g = W2r.reshape(KG, 4, H, 4, 32)                          # [g, a, i, b, c]
    W2p = W2g.transpose(0, 3, 1, 4, 2).reshape(64, P, H)        # [(g,b), (a,c), i]
    W2P_host = np.ascontiguousarray(W2p.transpose(1, 0, 2).astype(f16))  # [p, 64, i]
    W2b2_host = np.ascontiguousarray(np.asarray(b2, f32).reshape(H, H).T.astype(f16))
    W1p = np.concatenate([np.asarray(W1, f32), np.asarray(b1, f32)[None, :]], 0)

    W_ihT = np.ascontiguousarray(np.asarray(W_ih, f32).T)       # [128, 384]
    W_hhT = np.ascontiguousarray(np.asarray(W_hh, f32).T)
    b_ih = np.asarray(b_ih, f32)
    b_hh = np.asarray(b_hh, f32)
    b_r = (b_ih[:H] + b_hh[:H]).reshape(H, 1).astype(f32)
    b_z = (b_ih[H:2 * H] + b_hh[H:2 * H]).reshape(H, 1).astype(f32)
    b_in = b_ih[2 * H:].reshape(H, 1).astype(f32)
    b_hn = b_hh[2 * H:].reshape(H, 1).astype(f32)

    in_maps = []
    for c in range(NCORES):
        s, toff, ef = shards[c]
        n = len(s)
        s_pad = np.zeros(CAP, np.int32)
        s_pad[:n] = s
        ef_pad = np.zeros((CAP, ED), f32)
        ef_pad[:n] = ef
        efT = np.concatenate([ef_pad.T, np.ones((1, CAP), f32)], 0)   # [17, CAP]
        srcidx = np.ascontiguousarray(s_pad.reshape(ET, P).T)         # [128, ET]
        Sband = np.zeros((P, ET, W_band), f16)
        idx = np.arange(n)
        Sband[idx % P, idx // P, toff - base[idx // P]] = 1.0
        hTs = np.ascontiguousarray(h[c * NS:(c + 1) * NS].T)          # [128, 1024]
        in_maps.append(dict(
            hf16=hf16, efT=efT, srcidx=srcidx, Sband=Sband, W2P=W2P_host,
            W2b2=W2b2_host, W1p=W1p, WihT=W_ihT, WhhT=W_hhT, b_r=b_r,
            b_z=b_z, b_in=b_in, b_hn=b_hn, hTs=hTs))
    return (in_maps, CAP, W_band, tuple(int(b) for b in base),
            tuple(tuple(cc) for cc in contrib))


def _build_program(CAP, W_band, base_arr, contrib):
    import concourse.bass as bass
    import concourse.bacc as bacc
    import concourse.tile as tile
    import concourse.mybir as mybir
    from concourse.masks import make_identity

    dt = mybir.dt.float32
    f16 = mybir.dt.float16
    f32r = mybir.dt.float32r
    dti = mybir.dt.int32
    AF = mybir.ActivationFunctionType
    OP = mybir.AluOpType

    ET = CAP // P
    chunks = []
    t0 = 0
    while t0 < ET:
        nt_ = min(4, ET - t0)
        chunks.append((t0, nt_))
        t0 += nt_
    NCH = len(chunks)
    last_tile_of_chunk = [t0_ + nt_ - 1 for (t0_, nt_) in chunks]

    ready = [[] for _ in range(NCH)]
    empty_ng = []
    for ng in range(NT):
        if not contrib[ng]:
            empty_ng.append(ng)
            continue
        need = max(contrib[ng])
        for ci in range(NCH):
            if need <= last_tile_of_chunk[ci]:
                ready[ci].append(ng)
                break
    ngrp = GRU_COLS // P
    ng_done_at = {ng: 0 for ng in empty_ng}
    for ci in range(NCH):
        for ng in ready[ci]:
            ng_done_at[ng] = ci
    gru_ready = [[] for _ in range(NCH)]
    for gg in range(NS // GRU_COLS):
        ci = max(ng_done_at[gg * ngrp + i] for i in range(ngrp))
        gru_ready[ci].append(gg)

    nc = bacc.Bacc("TRN2", target_bir_lowering=False, debug=False,
                   num_devices=NCORES)

    hf16_d = nc.dram_tensor("hf16", [N, H], f16, kind="ExternalInput")
    efT_d = nc.dram_tensor("efT", [ED + 1, CAP], f32r, kind="ExternalInput")
    src_d = nc.dram_tensor("srcidx", [P, ET], dti, kind="ExternalInput")
    S_d = nc.dram_tensor("Sband", [P, ET, W_band], f16, kind="ExternalInput")
    W2P_d = nc.dram_tensor("W2P", [P, 64, H], f16, kind="ExternalInput")
    W2b2_d = nc.dram_tensor("W2b2", [P, H], f16, kind="ExternalInput")
    W1p_d = nc.dram_tensor("W1p", [ED + 1, MLP_HID], f32r, kind="ExternalInput")
    WihT_d = nc.dram_tensor("WihT", [H, 3 * H], f32r, kind="ExternalInput")
    WhhT_d = nc.dram_tensor("WhhT", [H, 3 * H], f32r, kind="ExternalInput")
    br_d = nc.dram_tensor("b_r", [H, 1], dt, kind="ExternalInput")
    bz_d = nc.dram_tensor("b_z", [H, 1], dt, kind="ExternalInput")
    bin_d = nc.dram_tensor("b_in", [H, 1], dt, kind="ExternalInput")
    bhn_d = nc.dram_tensor("b_hn", [H, 1], dt, kind="ExternalInput")
    hTs_d = nc.dram_tensor("hTs", [H, NS], f32r, kind="ExternalInput")
    out_d = nc.dram_tensor("out_hT", [H, NS], dt, kind="ExternalOutput")

    with tile.TileContext(nc) as tc:
        with (
            tc.tile_pool(name="const", bufs=1) as cp,
            tc.tile_pool(name="dram", bufs=1, space="DRAM") as dram,
            tc.tile_pool(name="work", bufs=1) as wp,
        ):
            # ---------- startup
            srci = cp.tile([P, ET], dti)
            nc.sync.dma_start(srci[:], src_d[:])
            ident16 = cp.tile([P, P], f16)
            make_identity(nc, ident16[:])
            efT = cp.tile([ED + 1, CAP], f32r)
            nc.scalar.dma_start(efT[:], efT_d[:])
            W1p = cp.tile([ED + 1, MLP_HID], f32r)
            nc.scalar.dma_start(W1p[:], W1p_d[:])

            hw16 = wp.tile([P, ET, P], f16)
            for t in range(ET):
                nc.gpsimd.indirect_dma_start(
                    out=hw16[:, t, :], out_offset=None, in_=hf16_d[:],
                    in_offset=bass.IndirectOffsetOnAxis(ap=srci[:, t:t + 1], axis=0))

            W2P = cp.tile([P, 64, H], f16)
            for q in range(4):
                nc.sync.dma_start(W2P[:, q * 16:(q + 1) * 16, :],
                                  W2P_d[:, q * 16:(q + 1) * 16, :])
            w2b2 = cp.tile([P, H], f16)
            nc.sync.dma_start(w2b2[:], W2b2_d[:])
            WihT = cp.tile([H, 3 * H], f32r)
            nc.sync.dma_start(WihT[:], WihT_d[:])
            WhhT = cp.tile([H, 3 * H], f32r)
            nc.sync.dma_start(WhhT[:], WhhT_d[:])
            b_r = cp.tile([H, 1], dt)
            nc.sync.dma_start(b_r[:], br_d[:])
            b_z = cp.tile([H, 1], dt)
            nc.sync.dma_start(b_z[:], bz_d[:])
            b_in = cp.tile([H, 1], dt)
            nc.sync.dma_start(b_in[:], bin_d[:])
            b_hn = cp.tile([H, 1], dt)
            nc.sync.dma_start(b_hn[:], bhn_d[:])
            hTs = cp.tile([H, NS], f32r)
            nc.sync.dma_start(hTs[:], hTs_d[:])

            Sband = cp.tile([P, ET, W_band], f16)
            for (t0_, nt_) in chunks[:1]:
                for t in range(t0_, t0_ + nt_):
                    nc.sync.dma_start(Sband[:, t, :], S_d[:, t, :])

            zT = wp.tile([MLP_HID, CAP], f16)
            zT_dram = dram.tile([MLP_HID, CAP], f16)
            hwT = wp.tile([P, CAP], f16)
            hwT_dram = dram.tile([P, CAP], f16)
            H32 = wp.tile([P, 4, CAP], f16)

            # Z chunk 0 first so chunk-0 z broadcasts can start early, then
            # the rest of Z, all on PE before the main loop.
            with tc.tile_pool(name="psz", bufs=2, space="PSUM") as psz:
                for (t0_, nt_) in chunks:
                    c0, cw = t0_ * P, nt_ * P
                    zps = psz.tile([MLP_HID, 512], dt, tag="zps")
                    nc.tensor.matmul(zps[:, :cw], W1p[:], efT[:, c0:c0 + cw],
                                     start=True, stop=True)
                    nc.scalar.activation(zT[:, c0:c0 + cw], zps[:, :cw], AF.Relu)
                    nc.scalar.dma_start(zT_dram[:, c0:c0 + cw],
                                        zT[:, c0:c0 + cw])

            def hw_transpose(t, pool):
                tp = pool.tile([P, P], f16, tag="tp")
                nc.tensor.transpose(tp[:], hw16[:, t, :], ident16[:])
                nc.scalar.copy(hwT[:, t * P:(t + 1) * P], tp[:])

            def emit_h32(ci):
                (t0_, nt_) = chunks[ci]
                c0, cw = t0_ * P, nt_ * P
                nc.scalar.dma_start(hwT_dram[:, c0:c0 + cw], hwT[:, c0:c0 + cw])
                src_view = hwT_dram[:, c0:c0 + cw].rearrange(
                    "(b c) e -> c b e", b=4)
                for a in range(4):
                    eng = (nc.scalar, nc.gpsimd)[a % 2] if ci else nc.scalar
                    eng.dma_start(H32[32 * a:32 * a + 32, :, c0:c0 + cw],
                                  src_view)

            def emit_z32(ci, slot):
                (t0_, nt_) = chunks[ci]
                c0, cw = t0_ * P, nt_ * P
                zv = zT_dram[:, c0:c0 + cw].rearrange("(g a) e -> a g e", a=4)
                for a in range(4):
                    eng = (nc.sync, nc.scalar)[a % 2]
                    eng.dma_start(
                        slot[32 * a:32 * a + 32, :, :cw],
                        zv[a:a + 1].broadcast_to((32, KG, cw)))

            # ---------- main pipeline
            msgT16 = wp.tile([P, CAP], f16)
            msg = wp.tile([P, ET, P], f16)
            mT = wp.tile([H, NS], f32r)
            out_sb = wp.tile([H, NS], dt)
            for ng in empty_ng:
                nc.gpsimd.memset(mT[:, ng * P:(ng + 1) * P], 0.0)

            with (
                tc.tile_pool(name="psacc", bufs=2, space="PSUM") as psacc,
                tc.tile_pool(name="pstr", bufs=2, space="PSUM") as pstr,
                tc.tile_pool(name="psm", bufs=2, space="PSUM") as psm,
                tc.tile_pool(name="psg", bufs=1, space="PSUM") as psg,
                tc.tile_pool(name="zpool", bufs=2) as zpool,
                tc.tile_pool(name="ptpool", bufs=3) as ptpool,
            ):
                def scat_ng(ng):
                    pm = psm.tile([P, P], dt, tag="pm")
                    for idx, ti in enumerate(contrib[ng]):
                        off = ng * P - int(base_arr[ti])
                        nc.tensor.matmul(
                            pm[:], msg[:, ti, :], Sband[:, ti, off:off + P],
                            start=(idx == 0),
                            stop=(idx == len(contrib[ng]) - 1))
                    nc.scalar.copy(mT[:, ng * P:(ng + 1) * P], pm[:])

                def msg_transpose(t):
                    tp = pstr.tile([P, P], f16, tag="tp")
                    nc.tensor.transpose(
                        tp[:], msgT16[:, t * P:(t + 1) * P], ident16[:])
                    nc.scalar.copy(msg[:, t, :], tp[:])

                def gru_group(gg):
                    c0 = gg * GRU_COLS
                    csl = slice(c0, c0 + GRU_COLS)
                    rz_ps = psg.tile([H, 2, GRU_COLS], dt, tag="rzp")
                    nn_ps = psg.tile([H, 2, GRU_COLS], dt, tag="nnp")
                    gin_ps = nn_ps[:, 0, :]
                    ghn_ps = nn_ps[:, 1, :]
                    for q in range(2):
                        nc.tensor.matmul(rz_ps[:, q, :],
                                         WihT[:, q * H:(q + 1) * H],
                                         mT[:, csl], start=True, stop=False)
                        nc.tensor.matmul(rz_ps[:, q, :],
                                         WhhT[:, q * H:(q + 1) * H],
                                         hTs[:, csl], start=False, stop=True)
                    nc.tensor.matmul(gin_ps, WihT[:, 2 * H:3 * H],
                                     mT[:, csl], start=True, stop=True)
                    nc.tensor.matmul(ghn_ps, WhhT[:, 2 * H:3 * H],
                                     hTs[:, csl], start=True, stop=True)
                    rz = wp.tile([H, 2, GRU_COLS], dt, tag=f"rz{gg % 2}")
                    nc.scalar.activation(rz[:, 0, :], rz_ps[:, 0, :],
                                         AF.Sigmoid, bias=b_r[:])
                    nc.scalar.activation(rz[:, 1, :], rz_ps[:, 1, :],
                                         AF.Sigmoid, bias=b_z[:])
                    ghn = wp.tile([H, GRU_COLS], dt, tag=f"ghn{gg % 2}")
                    nc.scalar.activation(ghn[:], ghn_ps, AF.Identity,
                                         bias=b_hn[:])
                    nc.gpsimd.tensor_mul(ghn[:], rz[:, 0, :], ghn[:])
                    nc.vector.tensor_add(ghn[:], ghn[:], gin_ps)
                    ng_ = wp.tile([H, GRU_COLS], dt, tag=f"ng{gg % 2}")
                    nc.scalar.activation(ng_[:], ghn[:], AF.Tanh, bias=b_in[:])
                    dif = wp.tile([H, GRU_COLS], dt, tag=f"dif{gg % 2}")
                    nc.gpsimd.tensor_sub(dif[:], hTs[:, csl].bitcast(dt), ng_[:])
                    nc.gpsimd.tensor_mul(dif[:], rz[:, 1, :], dif[:])
                    nc.gpsimd.tensor_add(out_sb[:, csl], ng_[:], dif[:])
                    nc.sync.dma_start(out_d[:, csl], out_sb[:, csl])

                # startup tail: transposes for chunk 0, its H32 + Z32
                zslot0 = zpool.tile([P, KG, 512], f16, tag="zs0")
                zslot1 = zpool.tile([P, KG, 512], f16, tag="zs1")
                zslots = [zslot0, zslot1]
                for t in range(chunks[0][0], chunks[0][0] + chunks[0][1]):
                    hw_transpose(t, pstr)
                emit_h32(0)
                emit_z32(0, zslots[0])

                deferred = []
                for ci, (t0_, nt_) in enumerate(chunks):
                    c0, cw = t0_ * P, nt_ * P
                    if ci + 1 < NCH:
                        emit_z32(ci + 1, zslots[(ci + 1) % 2])
                        for t in range(chunks[ci + 1][0],
                                       chunks[ci + 1][0] + chunks[ci + 1][1]):
                            nc.sync.dma_start(Sband[:, t, :], S_d[:, t, :])
                    acc = psacc.tile([P, 512], dt, tag="acc")
                    slot = zslots[ci % 2]
                    tslots = {2: 0, 6: 1, 10: 2, 14: 3}
                    for g in range(KG):
                        pt = ptpool.tile([P, 4, 512], f16, tag="pt")
                        nc.vector.tensor_tensor(
                            pt[:, :, :cw],
                            slot[:, g, :cw].unsqueeze(1)
                            .broadcast_to((P, 4, cw)),
                            H32[:, :, c0:c0 + cw], OP.mult)
                        for b in range(4):
                            nc.tensor.matmul(
                                acc[:, :cw], W2P[:, 4 * g + b, :],
                                pt[:, b, :cw],
                                start=(g == 0 and b == 0), stop=False)
                        # PE gap fillers: next chunk's h_w transposes at
                        # fixed slots, otherwise one deferred scatter/GRU op
                        filled = False
                        if g in tslots and ci + 1 < NCH:
                            t = chunks[ci + 1][0] + tslots[g]
                            if t < chunks[ci + 1][0] + chunks[ci + 1][1]:
                                hw_transpose(t, pstr)
                                filled = True
                        if not filled and deferred:
                            deferred.pop(0)()
                    nc.tensor.matmul(acc[:, :cw], w2b2[:],
                                     hwT[:, c0:c0 + cw],
                                     start=False, stop=True)
                    nc.scalar.copy(msgT16[:, c0:c0 + cw], acc[:, :cw])
                    if ci + 1 < NCH:
                        emit_h32(ci + 1)
                    for t in range(t0_, t0_ + nt_):
                        deferred.append(lambda t=t: msg_transpose(t))
                    deferred += [lambda ng=ng: scat_ng(ng) for ng in ready[ci]]
                    deferred += [lambda gg=gg: gru_group(gg)
                                 for gg in gru_ready[ci]]
                for fn in deferred:
                    fn()

    nc.compile()
    return nc


_CACHE = {}


def _get_program(CAP, W_band, base, contrib):
    key = (CAP, W_band, base, contrib)
    if key not in _CACHE:
        _CACHE[key] = _build_program(CAP, W_band, base, contrib)
    return _CACHE[key]


def kernel(h, edge_index, edge_features, W1, b1, W2, b2, W_ih, W_hh, b_ih, b_hh):
    from concourse import bass_utils

    in_maps, CAP, W_band, base, contrib = _host_prep(
        h, edge_index, edge_features, W1, b1, W2, b2, W_ih, W_hh, b_ih, b_hh)
    nc = _get_program(CAP, W_band, base, contrib)
    res = bass_utils.run_bass_kernel_spmd(nc, in_maps, core_ids=list(range(NCORES)))
    out = np.empty((N, H), np.float32)
    for c in range(NCORES):
        out[c * NS:(c + 1) * NS] = res.results[c]["out_hT"].T
    return out
